# revision 25
# baseline (speedup 1.0000x reference)
"""Trainium2 Bass kernel for nn_Attention_30760555774660 (stacked attention VQA).

Sharding: data-parallel over batch, 256 -> 8 cores x 32. Weights replicated.

Per-core structure (B=32, S=196, D=1024, A=512, O=3000):
  - img is shipped fp8e4 twice: pre-transposed imgT8 per-window tensors for
    the projections, and natural img8N [B, 98, 2*D] for the vI weighted sums.
  - Projection img @ W_ia runs s-flat in fp8 DoubleRow mode: 25 chunks of
    [<=128 s, 512 a] PSUM, 4 K-pair matmuls each (K=256 per matmul via
    DoubleRow), W_ia shipped fp8e4 scaled x32; plus a one-hot bf16 fold
    matmul adding the per-batch q-projection row (also scaled x32).
  - tanh on ScalarE with scale=1/32 (psum -> bf16 SBUF); logits via DVE
    mul+reduce against a partition-broadcast Wp.
  - Logit columns [128, 25] are PE-transposed then reshaped to [16, 196]
    via a DRAM round-trip on the sync DMA queue (the scalar queue is busy
    with tanhs); softmax is a 4-op sequence on 16 lanes.
  - vI in fp8 DoubleRow: diag-masked piT stationaries (pi x64 in fp8)
    against img8 [98, 2, D] tiles, accumulating all 16 batches into one
    PSUM [16, 512] pair. The x64 runs through the whole u-chain: ques is
    shipped x64, W_qa2 scaled x(32/64), W_fc x(1/64).
  - u1/u2 transposed once into u1T/u2T [128d, 32b] bf16 for the q-proj of
    block 2 and the final FC. W_fc tiles are prefetched mid-kernel so the
    fc tail is not DMA-bound; fc runs split per half, with fc(h0) covering
    the softmax(1,1) round-trip.
  - The two 16-batch halves are interleaved so softmax/DVE phases of one
    half hide under the other half's projection matmuls. Dummy bf16 warm
    matmuls (from a memset tile, no DMA dep) ramp the PE p-state at start.
"""

import os
import sys

import numpy as np

if "/opt/trn_rl_repo" not in sys.path:
    sys.path.insert(0, "/opt/trn_rl_repo")

B_FULL = 256
N_CORES = 8
B = B_FULL // N_CORES  # 32
BH = 16  # half-batch
S = 196
SG = S // 2  # 98, DoubleRow halves for vI
D = 1024
A = 512
O = 3000
SH = BH * S  # 3136 flat s-cols per half
DC = D // 128  # 8
OC = 6
ON = O // OC  # 500
WSCALE = 32.0
PISCALE = 64.0
# flat s-chunks per half: 24 x 128 + 1 x 64
CHUNKS = [(j * 128, 128) for j in range(24)] + [(3072, 64)]
# imgT8 load windows (s-cols) per half; first early so proj can start
WINDOWS = [(0, 1024), (1024, 1024), (2048, 1088)]
EARLY_W = [0, 1]  # covers chunks 0..15
LATE_W = [2]

_nc_cache = None


def _build_nc():
    import concourse.bacc as bacc
    import concourse.tile as tile
    from concourse import mybir
    import bass_rust  # noqa: F401
    import concourse.bass as bass

    f32 = mybir.dt.float32
    bf16 = mybir.dt.bfloat16
    f8 = mybir.dt.float8e4
    Tanh = mybir.ActivationFunctionType.Tanh
    Exp = mybir.ActivationFunctionType.Exp
    mult = mybir.AluOpType.mult
    add = mybir.AluOpType.add
    DR = mybir.MatmulPerfMode.DoubleRow

    nc = bacc.Bacc("TRN2", target_bir_lowering=False)

    img8_h = nc.dram_tensor("img8N", [B, SG, 2 * D], f8, kind="ExternalInput")
    imgtW_h = [
        nc.dram_tensor(f"imgT8W{w}", [2, 128, DC * wl], f8, kind="ExternalInput")
        for w, (w0, wl) in enumerate(WINDOWS)
    ]
    ques_h = nc.dram_tensor("ques", [B, D], f32, kind="ExternalInput")
    w8_1_h = nc.dram_tensor("W8_1", [128, DC, A], f8, kind="ExternalInput")
    qp1_h = nc.dram_tensor("QP1H", [B, A], bf16, kind="ExternalInput")
    wp1_h = nc.dram_tensor("Wp1", [A], bf16, kind="ExternalInput")
    w8_2_h = nc.dram_tensor("W8_2", [128, DC, A], f8, kind="ExternalInput")
    wqa2_h = nc.dram_tensor("WQA2", [128, DC, A], bf16, kind="ExternalInput")
    bqa2_h = nc.dram_tensor("b_qa2", [A], f32, kind="ExternalInput")
    wp2_h = nc.dram_tensor("Wp2", [A], bf16, kind="ExternalInput")
    wfc_h = nc.dram_tensor("WFC", [OC, 128, DC * ON], bf16, kind="ExternalInput")
    bfc_h = nc.dram_tensor("b_fc", [O], f32, kind="ExternalInput")
    sel_h = nc.dram_tensor("SEL", [BH, SH], bf16, kind="ExternalInput")
    identf_h = nc.dram_tensor("IDENTF", [128, 128], f32, kind="ExternalInput")
    score_h = nc.dram_tensor("score", [B, O], f32, kind="ExternalOutput")
    lcscr_h = nc.dram_tensor("LCSCR", [4, 3200], f32, kind="Internal")

    def bcast_ap(h, n_part, off=0, n=None):
        ap = h[off : off + n] if n is not None else h[:]
        return bass.AP(tensor=ap.tensor, offset=ap.offset, ap=[[0, n_part]] + ap.ap)

    def diag_ap(t_ap, npart, nb):
        # t_ap: AP [128, nb, nb]; view [npart, nb] hitting [p, i, i]
        pstride = t_ap.ap[0][0]
        return bass.AP(
            tensor=t_ap.tensor, offset=t_ap.offset, ap=[[pstride, npart], [nb + 1, nb]]
        )

    with tile.TileContext(nc) as tc:
        with (
            tc.tile_pool(name="const", bufs=1) as const,
            tc.tile_pool(name="imgt", bufs=2) as imgt_p,
            tc.tile_pool(name="imgt0", bufs=2) as imgt0_p,
            tc.tile_pool(name="imgn", bufs=16) as imgn_p,
            tc.tile_pool(name="wst", bufs=1) as wst,
            tc.tile_pool(name="ha", bufs=3) as ha_p,
            tc.tile_pool(name="lc", bufs=2) as lc_p,
            tc.tile_pool(name="work", bufs=2) as work,
            tc.tile_pool(name="uh", bufs=1) as uh_p,
            tc.tile_pool(name="psp", bufs=5, space="PSUM") as psp,
            tc.tile_pool(name="pst", bufs=2, space="PSUM") as pst,
        ):
            # ---------------- prologue, ordered by first use ----------------
            # warm source: memset, no DMA dependency
            wsrc = const.tile([128, 128], bf16)
            nc.vector.memset(wsrc[:, :].bitcast(f32), 0.0)

            # sync queue: QP1 halves -> imgT8 h0 windows -> ques -> identf
            QP1 = {}
            for h in range(2):
                qp = const.tile([BH, A], bf16, tag=f"QP1{h}")
                nc.sync.dma_start(out=qp, in_=qp1_h[h * BH : (h + 1) * BH, :])
                QP1[h] = qp

            imgT = {}

            def load_imgT(h, windows, split0=False):
                for w in windows:
                    w0, wl = WINDOWS[w]
                    pool = imgt0_p if w in EARLY_W else imgt_p
                    t = pool.tile([128, DC, wl], f8, tag=f"imgt_{w}")
                    imgT[(h, w)] = t
                    src_ap = imgtW_h[w][h, :, :].rearrange("p (c x) -> p c x", c=DC)
                    if split0 and w == 0:
                        nc.sync.dma_start(out=t[0:64, :, :], in_=src_ap[0:64, :, :])
                        nc.scalar.dma_start(out=t[64:128, :, :], in_=src_ap[64:128, :, :])
                    else:
                        nc.sync.dma_start(out=t, in_=src_ap)

            load_imgT(0, EARLY_W, split0=True)

            # gpsimd queue: wia1/sel/wp1b first (chunk 0), then the rest by need
            wia1 = const.tile([128, DC, A], f8)
            nc.gpsimd.dma_start(out=wia1, in_=w8_1_h[:, :, :])
            sel = const.tile([BH, SH], bf16)
            nc.gpsimd.dma_start(out=sel, in_=sel_h[:, :])
            wp1b = const.tile([128, A], bf16)
            nc.gpsimd.dma_start(out=wp1b, in_=bcast_ap(wp1_h, 128))

            load_imgT(0, LATE_W)

            quesA = {}
            for h in range(2):
                qa = const.tile([BH, D], f32, tag=f"quesA{h}")
                nc.sync.dma_start(out=qa, in_=ques_h[h * BH : (h + 1) * BH, :])
                quesA[h] = qa
            identf = const.tile([128, 128], f32)
            nc.sync.dma_start(out=identf, in_=identf_h[:, :])

            wqa2 = const.tile([128, DC, A], bf16)
            nc.gpsimd.dma_start(out=wqa2, in_=wqa2_h[:, :, :])
            bqa2b = const.tile([BH, A], f32)
            nc.gpsimd.dma_start(out=bqa2b, in_=bcast_ap(bqa2_h, BH))
            wia2 = const.tile([128, DC, A], f8)
            nc.gpsimd.dma_start(out=wia2, in_=w8_2_h[:, :, :])
            wp2b = const.tile([128, A], bf16)
            nc.gpsimd.dma_start(out=wp2b, in_=bcast_ap(wp2_h, 128))

            # masks for vI: [s-part(98), g, b, b] diag tiles in fp8, memset once
            mask8 = const.tile([SG, 2, BH, BH], f8)
            nc.vector.memset(mask8[:, :, :, :].bitcast(f32), 0.0)

            u1T = const.tile([128, DC, B], bf16)
            u2T = const.tile([128, DC, B], bf16)

            def imgT_chunk(h, s0, sl):
                for w, (w0, wl) in enumerate(WINDOWS):
                    if w0 <= s0 < w0 + wl:
                        return imgT[(h, w)], s0 - w0
                raise AssertionError(s0)

            # ---------------- phase helpers ----------------
            Lc = {}

            def proj(h, blk, lo=0, hi=len(CHUNKS)):
                """Projection chunks [lo,hi) for half h, block blk."""
                wia = wia1 if blk == 0 else wia2
                wpb = wp1b if blk == 0 else wp2b
                qp = QP1[h] if blk == 0 else QP2[h]
                if (h, blk) not in Lc:
                    lc = lc_p.tile([128, len(CHUNKS)], f32, tag=f"lc{h}{blk}")
                    Lc[(h, blk)] = lc
                lc = Lc[(h, blk)]
                for j in range(lo, hi):
                    s0, sl = CHUNKS[j]
                    t, off = imgT_chunk(h, s0, sl)
                    pp = psp.tile([128, A], f32, tag="pp")
                    for c in range(DC // 2):
                        nc.tensor.matmul(
                            pp[0:sl, :],
                            t[:, 2 * c : 2 * c + 2, off : off + sl],
                            wia[:, 2 * c : 2 * c + 2, :],
                            start=(c == 0),
                            stop=False,
                            perf_mode=DR,
                        )
                    nc.tensor.matmul(
                        pp[0:sl, :], sel[:, s0 : s0 + sl], qp, start=False, stop=True
                    )
                    ha = ha_p.tile([128, A], bf16, tag="ha")
                    nc.scalar.activation(ha[0:sl], pp[0:sl], Tanh, scale=1.0 / WSCALE)
                    tt = ha_p.tile([128, A], bf16, tag="tt")
                    nc.vector.tensor_mul(tt[0:sl], ha[0:sl], wpb[0:sl])
                    nc.vector.tensor_reduce(
                        lc[0:sl, j : j + 1],
                        tt[0:sl],
                        axis=mybir.AxisListType.X,
                        op=add,
                    )

            def warm(n):
                for _ in range(n):
                    jp = pst.tile([128, 128], f32, tag="tr")
                    nc.tensor.matmul(jp, wsrc, wsrc, start=True, stop=True)

            PIs = {}

            def softmax_pi(h, blk, q=None):
                """Lc[(h,blk)] -> PI [16, 196] fp32, scaled x64."""
                q = q if q is not None else nc.sync
                lc = Lc[(h, blk)]
                pt = pst.tile([len(CHUNKS), 128], f32, tag="tr")
                nc.tensor.transpose(pt, lc, identf)
                lcT = work.tile([len(CHUNKS), 128], f32, tag="lcT")
                nc.vector.tensor_copy(lcT, pt)
                idx = h * 2 + blk
                q.dma_start(out=lcscr_h[idx, 0:3200], in_=lcT[:, :])
                LT = work.tile([BH, S], f32, tag="LT")
                q.dma_start(
                    out=LT,
                    in_=lcscr_h[idx, 0:SH].rearrange("(b s) -> b s", b=BH),
                )
                E = work.tile([BH, S], f32, tag="E")
                Z = work.tile([BH, 1], f32, tag="Z")
                nc.scalar.activation(E, LT, Exp, accum_out=Z)
                R = work.tile([BH, 1], f32, tag="R")
                nc.vector.reciprocal(R, Z)
                R64 = work.tile([BH, 1], f32, tag="R64")
                nc.vector.tensor_scalar_mul(R64, R, PISCALE)
                PI = work.tile([BH, S], f32, tag="PI")
                nc.vector.tensor_scalar_mul(PI, E, R64)
                PIs[(h, blk)] = PI

            def make_masks(h, blk):
                PI = PIs[(h, blk)]
                for g in range(2):
                    pa = pst.tile([128, BH], f32, tag="tr")
                    nc.tensor.transpose(
                        pa[0:SG, :], PI[:, g * SG : (g + 1) * SG], identf[0:BH, 0:BH]
                    )
                    nc.vector.tensor_copy(
                        diag_ap(mask8[:, g, :, :], SG, BH), pa[0:SG, :]
                    )

            def vI_u(h, blk):
                """vI psum pair; u = vI + (ques | u1); returns u tile [16, 1024] f32."""
                vp0 = psp.tile([BH, A], f32, tag="pp")
                vp1 = psp.tile([BH, A], f32, tag="pp")
                vps = [vp0, vp1]
                for b in range(BH):
                    for dh in range(2):
                        inb = imgN[(h, blk, b)]
                        nc.tensor.matmul(
                            vps[dh],
                            mask8[:, :, b, :],
                            inb[:, :, dh * A : (dh + 1) * A],
                            start=(b == 0),
                            stop=(b == BH - 1),
                            perf_mode=DR,
                        )
                if blk == 0:
                    u = uh_p.tile([BH, D], f32, tag=f"u1_{h}")
                    other = quesA[h]
                else:
                    u = U1[h]
                    other = u
                for dh in range(2):
                    nc.vector.tensor_add(
                        u[:, dh * A : (dh + 1) * A], vps[dh], other[:, dh * A : (dh + 1) * A]
                    )
                return u

            def u_transpose(u, uT, h, idx):
                for c in range(DC):
                    pt = pst.tile([128, BH], f32, tag="tr")
                    nc.tensor.transpose(pt, u[:, c * 128 : (c + 1) * 128], identf[0:BH, 0:BH])
                    nc.vector.tensor_copy(uT[:, c, h * BH : (h + 1) * BH], pt)

            def qp2(h):
                qp_ps = psp.tile([BH, A], f32, tag="pp")
                for c in range(DC):
                    nc.tensor.matmul(
                        qp_ps,
                        u1T[:, c, h * BH : (h + 1) * BH],
                        wqa2[:, c, :],
                        start=(c == 0),
                        stop=(c == DC - 1),
                    )
                qp = const.tile([BH, A], bf16, tag=f"QP2{h}")
                nc.vector.tensor_add(qp, qp_ps, bqa2b)
                QP2[h] = qp

            def load_imgN(h, blk):
                for b in range(BH):
                    gb = h * BH + b
                    inb = imgn_p.tile([SG, 2, D], f8, tag="imgn")
                    imgN[(h, blk, b)] = inb
                    nc.gpsimd.dma_start(
                        out=inb,
                        in_=img8_h[gb, :, :].rearrange("p (g d) -> p g d", g=2),
                    )

            WF = {}

            def prefetch_fc():
                bfS = const.tile([BH, O], f32)
                nc.gpsimd.dma_start(out=bfS, in_=bcast_ap(bfc_h, BH))
                WF["b"] = bfS
                for n in range(OC):
                    wf = wst.tile([128, DC, ON], bf16, tag=f"wf{n}")
                    nc.sync.dma_start(
                        out=wf,
                        in_=wfc_h[n, :, :].rearrange("p (c x) -> p c x", c=DC),
                    )
                    WF[n] = wf

            def fc(h):
                bfS = WF["b"]
                for n in range(OC):
                    wf = WF[n]
                    fp = psp.tile([BH, ON], f32, tag="pp")
                    for c in range(DC):
                        nc.tensor.matmul(
                            fp,
                            u2T[:, c, h * BH : (h + 1) * BH],
                            wf[:, c, :],
                            start=(c == 0),
                            stop=(c == DC - 1),
                        )
                    sc = work.tile([BH, ON], f32, tag="sc")
                    nc.vector.tensor_add(sc, fp, bfS[:, n * ON : (n + 1) * ON])
                    nc.sync.dma_start(
                        out=score_h[h * BH : (h + 1) * BH, n * ON : (n + 1) * ON],
                        in_=sc,
                    )

            # -------- main schedule: halves interleaved to hide softmax --------
            imgN = {}
            U1 = {}
            QP2 = {}

            load_imgN(0, 0)
            warm(48)
            proj(0, 0)

            load_imgT(1, EARLY_W)
            softmax_pi(0, 0)
            make_masks(0, 0)
            proj(1, 0, 0, 16)  # hides softmax(0,0); needs h1 windows 0-1

            U1[0] = vI_u(0, 0)
            u_transpose(U1[0], u1T, 0, 0)
            qp2(0)
            load_imgN(0, 1)
            proj(0, 1)

            softmax_pi(0, 1)  # issue before wfc hits the sync queue
            load_imgT(1, LATE_W)
            prefetch_fc()
            load_imgN(1, 0)
            proj(1, 0, 16, len(CHUNKS))
            softmax_pi(1, 0, nc.scalar)  # scalar q is idle after proj tanhs
            make_masks(0, 1)

            u2_0 = vI_u(0, 1)
            u_transpose(u2_0, u2T, 0, 2)
            make_masks(1, 0)

            U1[1] = vI_u(1, 0)
            u_transpose(U1[1], u1T, 1, 1)
            qp2(1)
            load_imgN(1, 1)
            proj(1, 1)

            softmax_pi(1, 1, nc.scalar)
            fc(0)  # h0 classifier hides softmax(1,1) round-trip
            make_masks(1, 1)

            u2_1 = vI_u(1, 1)
            u_transpose(u2_1, u2T, 1, 3)
            fc(1)

    nc.compile()
    return nc


def _get_nc():
    global _nc_cache
    if _nc_cache is None:
        _nc_cache = _build_nc()
    return _nc_cache


def _np_cast(x, dt):
    import jax

    x = np.asarray(x)
    if x.nbytes >= 1 << 22:
        # big tensors: multithreaded conversion via jax CPU
        cpu = jax.devices("cpu")[0]
        with jax.default_device(cpu):
            y = jax.jit(lambda v: v.astype(dt), backend="cpu")(x)
            return np.asarray(y)
    return x.astype(dt)


def _to_bf16(x):
    import ml_dtypes

    return _np_cast(x, ml_dtypes.bfloat16)


def _make_in_maps(inputs):
    import ml_dtypes

    bf = ml_dtypes.bfloat16
    f8 = ml_dtypes.float8_e4m3
    ident = np.eye(128)
    selmat = np.zeros((BH, SH), np.float32)
    for b in range(BH):
        selmat[b, b * S : (b + 1) * S] = 1.0
    img_f8 = _np_cast(inputs["img_feat"], f8)
    qp1_full = (
        WSCALE
        * (
            np.asarray(inputs["ques_feat"], np.float32).astype(bf).astype(np.float32)
            @ np.asarray(inputs["W_qa1"], np.float32).astype(bf).astype(np.float32)
            + np.asarray(inputs["b_qa1"], np.float32)
        )
    ).astype(bf)

    def warr(w, dt, scale=1.0):
        # [D, X] -> [128, DC, X] (p, c, x) with d = c*128 + p
        w = np.asarray(w, np.float32) * scale
        x = w.shape[1]
        return np.ascontiguousarray(
            w.reshape(DC, 128, x).transpose(1, 0, 2)
        ).astype(dt)

    shared = {
        "W8_1": warr(inputs["W_ia1"], f8, WSCALE),
        "Wp1": np.asarray(inputs["Wp1"]).astype(bf),
        "W8_2": warr(inputs["W_ia2"], f8, WSCALE),
        "WQA2": warr(inputs["W_qa2"], bf, WSCALE / PISCALE),
        "b_qa2": np.ascontiguousarray(WSCALE * np.asarray(inputs["b_qa2"], np.float32)),
        "Wp2": np.asarray(inputs["Wp2"]).astype(bf),
        "WFC": np.ascontiguousarray(
            warr(inputs["W_fc"], np.float32, 1.0 / PISCALE)
            .reshape(128, DC, OC, ON)
            .transpose(2, 0, 1, 3)
        ).reshape(OC, 128, DC * ON).astype(bf),
        "b_fc": np.ascontiguousarray(inputs["b_fc"], np.float32),
        "SEL": selmat.astype(bf),
        "IDENTF": ident.astype(np.float32),
    }
    in_maps = []
    for c in range(N_CORES):
        sl = slice(c * B, (c + 1) * B)
        m = dict(shared)
        m["img8N"] = np.ascontiguousarray(
            img_f8[sl].reshape(B, 2, SG, D).transpose(0, 2, 1, 3).reshape(B, SG, 2 * D)
        )
        # imgT: (h, p, cc, s) with d = cc*128 + p, split per window contiguous
        imgT = (
            img_f8[sl]
            .reshape(2, SH, D)
            .transpose(0, 2, 1)
            .reshape(2, DC, 128, SH)
            .transpose(0, 2, 1, 3)
        )
        for w, (w0, wl) in enumerate(WINDOWS):
            m[f"imgT8W{w}"] = np.ascontiguousarray(
                imgT[:, :, :, w0 : w0 + wl]
            ).reshape(2, 128, DC * wl)
        m["ques"] = np.ascontiguousarray(
            PISCALE * np.asarray(inputs["ques_feat"], np.float32)[sl]
        )
        m["QP1H"] = qp1_full[sl]
        in_maps.append(m)
    return in_maps


def kernel_run(inputs, trace=False):
    from concourse.bass_utils import run_bass_kernel_spmd

    nc = _get_nc()
    in_maps = _make_in_maps(inputs)
    res = run_bass_kernel_spmd(nc, in_maps, core_ids=list(range(N_CORES)), trace=trace)
    out = np.concatenate([r["score"] for r in res.results], axis=0)
    return out, res


def kernel(**inputs):
    out, _ = kernel_run(inputs)
    return out


# revision 27
# speedup vs baseline: 25051.1046x; 25051.1046x over previous
"""Trainium2 Bass kernel for nn_Attention_30760555774660 (stacked attention VQA).

Sharding: data-parallel over batch, 256 -> 8 cores x 32. Weights replicated.

Per-core structure (B=32, S=196, D=1024, A=512, O=3000):
  - img is shipped fp8e4 twice: pre-transposed imgT8 per-window tensors for
    the projections, and natural img8N [B, 98, 2*D] for the vI weighted sums.
  - Projection img @ W_ia runs s-flat in fp8 DoubleRow mode: 25 chunks of
    [<=128 s, 512 a] PSUM, 4 K-pair matmuls each (K=256 per matmul via
    DoubleRow), W_ia shipped fp8e4 scaled x32; plus a one-hot bf16 fold
    matmul adding the per-batch q-projection row (also scaled x32).
  - tanh on ScalarE with scale=1/32 (psum -> bf16 SBUF); logits via DVE
    mul+reduce against a partition-broadcast Wp.
  - Logit columns [128, 25] are PE-transposed then reshaped to [16, 196]
    via a DRAM round-trip on the sync DMA queue (the scalar queue is busy
    with tanhs); softmax is a 4-op sequence on 16 lanes.
  - vI in fp8 DoubleRow: diag-masked piT stationaries (pi x64 in fp8)
    against img8 [98, 2, D] tiles, accumulating all 16 batches into one
    PSUM [16, 512] pair. The x64 runs through the whole u-chain: ques is
    shipped x64, W_qa2 scaled x(32/64), W_fc x(1/64).
  - u1/u2 transposed once into u1T/u2T [128d, 32b] bf16 for the q-proj of
    block 2 and the final FC. W_fc tiles are prefetched mid-kernel so the
    fc tail is not DMA-bound; fc runs split per half, with fc(h0) covering
    the softmax(1,1) round-trip.
  - The two 16-batch halves are interleaved so softmax/DVE phases of one
    half hide under the other half's projection matmuls. Dummy bf16 warm
    matmuls (from a memset tile, no DMA dep) ramp the PE p-state at start.
"""

import os
import sys

import numpy as np

if "/opt/trn_rl_repo" not in sys.path:
    sys.path.insert(0, "/opt/trn_rl_repo")

B_FULL = 256
N_CORES = 8
B = B_FULL // N_CORES  # 32
BH = 16  # half-batch
S = 196
SG = S // 2  # 98, DoubleRow halves for vI
D = 1024
A = 512
O = 3000
SH = BH * S  # 3136 flat s-cols per half
DC = D // 128  # 8
OC = 6
ON = O // OC  # 500
WSCALE = 32.0
PISCALE = 64.0
# flat s-chunks per half: 24 x 128 + 1 x 64
CHUNKS = [(j * 128, 128) for j in range(24)] + [(3072, 64)]
# imgT8 load windows (s-cols) per half; first early so proj can start
WINDOWS = [(0, 1024), (1024, 1024), (2048, 1088)]
EARLY_W = [0, 1]  # covers chunks 0..15
LATE_W = [2]

_nc_cache = None


def _build_nc():
    import concourse.bacc as bacc
    import concourse.tile as tile
    from concourse import mybir
    import bass_rust  # noqa: F401
    import concourse.bass as bass

    f32 = mybir.dt.float32
    bf16 = mybir.dt.bfloat16
    f8 = mybir.dt.float8e4
    Tanh = mybir.ActivationFunctionType.Tanh
    Exp = mybir.ActivationFunctionType.Exp
    mult = mybir.AluOpType.mult
    add = mybir.AluOpType.add
    DR = mybir.MatmulPerfMode.DoubleRow

    nc = bacc.Bacc("TRN2", target_bir_lowering=False)

    img8_h = nc.dram_tensor("img8N", [B, SG, 2 * D], f8, kind="ExternalInput")
    imgtW_h = [
        nc.dram_tensor(f"imgT8W{w}", [2, 128, DC * wl], f8, kind="ExternalInput")
        for w, (w0, wl) in enumerate(WINDOWS)
    ]
    ques_h = nc.dram_tensor("ques", [B, D], f32, kind="ExternalInput")
    w8_1_h = nc.dram_tensor("W8_1", [128, DC, A], f8, kind="ExternalInput")
    qp1_h = nc.dram_tensor("QP1H", [B, A], bf16, kind="ExternalInput")
    wp1_h = nc.dram_tensor("Wp1", [A], bf16, kind="ExternalInput")
    w8_2_h = nc.dram_tensor("W8_2", [128, DC, A], f8, kind="ExternalInput")
    wqa2_h = nc.dram_tensor("WQA2", [128, DC, A], bf16, kind="ExternalInput")
    bqa2_h = nc.dram_tensor("b_qa2", [A], f32, kind="ExternalInput")
    wp2_h = nc.dram_tensor("Wp2", [A], bf16, kind="ExternalInput")
    wfc_h = nc.dram_tensor("WFC", [OC, 128, DC * ON], bf16, kind="ExternalInput")
    bfc_h = nc.dram_tensor("b_fc", [O], f32, kind="ExternalInput")
    sel_h = nc.dram_tensor("SEL", [BH, SH], bf16, kind="ExternalInput")
    identf_h = nc.dram_tensor("IDENTF", [128, 128], f32, kind="ExternalInput")
    score_h = nc.dram_tensor("score", [B, O], f32, kind="ExternalOutput")
    lcscr_h = nc.dram_tensor("LCSCR", [4, 3200], f32, kind="Internal")

    def bcast_ap(h, n_part, off=0, n=None):
        ap = h[off : off + n] if n is not None else h[:]
        return bass.AP(tensor=ap.tensor, offset=ap.offset, ap=[[0, n_part]] + ap.ap)

    def diag_ap(t_ap, npart, nb):
        # t_ap: AP [128, nb, nb]; view [npart, nb] hitting [p, i, i]
        pstride = t_ap.ap[0][0]
        return bass.AP(
            tensor=t_ap.tensor, offset=t_ap.offset, ap=[[pstride, npart], [nb + 1, nb]]
        )

    with tile.TileContext(nc) as tc:
        with (
            tc.tile_pool(name="const", bufs=1) as const,
            tc.tile_pool(name="imgt", bufs=2) as imgt_p,
            tc.tile_pool(name="imgt0", bufs=2) as imgt0_p,
            tc.tile_pool(name="imgn", bufs=16) as imgn_p,
            tc.tile_pool(name="wst", bufs=1) as wst,
            tc.tile_pool(name="ha", bufs=3) as ha_p,
            tc.tile_pool(name="lc", bufs=2) as lc_p,
            tc.tile_pool(name="work", bufs=2) as work,
            tc.tile_pool(name="uh", bufs=1) as uh_p,
            tc.tile_pool(name="psp", bufs=5, space="PSUM") as psp,
            tc.tile_pool(name="pst", bufs=2, space="PSUM") as pst,
        ):
            # ---------------- prologue, ordered by first use ----------------
            # warm source: memset, no DMA dependency
            wsrc = const.tile([128, 128], bf16)
            nc.vector.memset(wsrc[:, :].bitcast(f32), 0.0)

            # sync queue: QP1 halves -> imgT8 h0 windows -> ques -> identf
            QP1 = {}
            for h in range(2):
                qp = const.tile([BH, A], bf16, tag=f"QP1{h}")
                nc.sync.dma_start(out=qp, in_=qp1_h[h * BH : (h + 1) * BH, :])
                QP1[h] = qp

            imgT = {}

            def load_imgT(h, windows, split0=False):
                for w in windows:
                    w0, wl = WINDOWS[w]
                    pool = imgt0_p if w in EARLY_W else imgt_p
                    t = pool.tile([128, DC, wl], f8, tag=f"imgt_{w}")
                    imgT[(h, w)] = t
                    src_ap = imgtW_h[w][h, :, :].rearrange("p (c x) -> p c x", c=DC)
                    if split0 and w == 0:
                        nc.sync.dma_start(out=t[0:64, :, :], in_=src_ap[0:64, :, :])
                        nc.scalar.dma_start(out=t[64:128, :, :], in_=src_ap[64:128, :, :])
                    else:
                        nc.sync.dma_start(out=t, in_=src_ap)

            load_imgT(0, EARLY_W, split0=True)

            # gpsimd queue: wia1/sel/wp1b first (chunk 0), then the rest by need
            wia1 = const.tile([128, DC, A], f8)
            nc.gpsimd.dma_start(out=wia1, in_=w8_1_h[:, :, :])
            sel = const.tile([BH, SH], bf16)
            nc.gpsimd.dma_start(out=sel, in_=sel_h[:, :])
            wp1b = const.tile([128, A], bf16)
            nc.gpsimd.dma_start(out=wp1b, in_=bcast_ap(wp1_h, 128))

            load_imgT(0, LATE_W)

            quesA = {}
            for h in range(2):
                qa = const.tile([BH, D], f32, tag=f"quesA{h}")
                nc.sync.dma_start(out=qa, in_=ques_h[h * BH : (h + 1) * BH, :])
                quesA[h] = qa
            identf = const.tile([128, 128], f32)
            nc.sync.dma_start(out=identf, in_=identf_h[:, :])

            wqa2 = const.tile([128, DC, A], bf16)
            nc.gpsimd.dma_start(out=wqa2, in_=wqa2_h[:, :, :])
            bqa2b = const.tile([BH, A], f32)
            nc.gpsimd.dma_start(out=bqa2b, in_=bcast_ap(bqa2_h, BH))
            wia2 = const.tile([128, DC, A], f8)
            nc.gpsimd.dma_start(out=wia2, in_=w8_2_h[:, :, :])
            wp2b = const.tile([128, A], bf16)
            nc.gpsimd.dma_start(out=wp2b, in_=bcast_ap(wp2_h, 128))

            # masks for vI: [s-part(98), g, b, b] diag tiles in fp8, memset once
            mask8 = const.tile([SG, 2, BH, BH], f8)
            nc.vector.memset(mask8[:, :, :, :].bitcast(f32), 0.0)

            u1T = const.tile([128, DC, B], bf16)
            u2T = const.tile([128, DC, B], bf16)

            def imgT_chunk(h, s0, sl):
                for w, (w0, wl) in enumerate(WINDOWS):
                    if w0 <= s0 < w0 + wl:
                        return imgT[(h, w)], s0 - w0
                raise AssertionError(s0)

            # ---------------- phase helpers ----------------
            Lc = {}

            def proj(h, blk, lo=0, hi=len(CHUNKS)):
                """Projection chunks [lo,hi) for half h, block blk."""
                wia = wia1 if blk == 0 else wia2
                wpb = wp1b if blk == 0 else wp2b
                qp = QP1[h] if blk == 0 else QP2[h]
                if (h, blk) not in Lc:
                    lc = lc_p.tile([128, len(CHUNKS)], f32, tag=f"lc{h}{blk}")
                    Lc[(h, blk)] = lc
                lc = Lc[(h, blk)]
                for j in range(lo, hi):
                    s0, sl = CHUNKS[j]
                    t, off = imgT_chunk(h, s0, sl)
                    pp = psp.tile([128, A], f32, tag="pp")
                    for c in range(DC // 2):
                        nc.tensor.matmul(
                            pp[0:sl, :],
                            t[:, 2 * c : 2 * c + 2, off : off + sl],
                            wia[:, 2 * c : 2 * c + 2, :],
                            start=(c == 0),
                            stop=False,
                            perf_mode=DR,
                        )
                    nc.tensor.matmul(
                        pp[0:sl, :], sel[:, s0 : s0 + sl], qp, start=False, stop=True
                    )
                    ha = ha_p.tile([128, A], bf16, tag="ha")
                    nc.scalar.activation(ha[0:sl], pp[0:sl], Tanh, scale=1.0 / WSCALE)
                    tt = ha_p.tile([128, A], bf16, tag="tt")
                    nc.vector.tensor_mul(tt[0:sl], ha[0:sl], wpb[0:sl])
                    nc.vector.tensor_reduce(
                        lc[0:sl, j : j + 1],
                        tt[0:sl],
                        axis=mybir.AxisListType.X,
                        op=add,
                    )

            def warm(n):
                for _ in range(n):
                    jp = pst.tile([128, 128], f32, tag="tr")
                    nc.tensor.matmul(jp, wsrc, wsrc, start=True, stop=True)

            PIs = {}

            def softmax_pi(h, blk, q=None):
                """Lc[(h,blk)] -> PI [16, 196] fp32, scaled x64."""
                q = q if q is not None else nc.sync
                lc = Lc[(h, blk)]
                pt = pst.tile([len(CHUNKS), 128], f32, tag="tr")
                nc.tensor.transpose(pt, lc, identf)
                lcT = work.tile([len(CHUNKS), 128], f32, tag="lcT")
                nc.vector.tensor_copy(lcT, pt)
                idx = h * 2 + blk
                q.dma_start(out=lcscr_h[idx, 0:3200], in_=lcT[:, :])
                LT = work.tile([BH, S], f32, tag="LT")
                q.dma_start(
                    out=LT,
                    in_=lcscr_h[idx, 0:SH].rearrange("(b s) -> b s", b=BH),
                )
                E = work.tile([BH, S], f32, tag="E")
                Z = work.tile([BH, 1], f32, tag="Z")
                nc.scalar.activation(E, LT, Exp, accum_out=Z)
                R = work.tile([BH, 1], f32, tag="R")
                nc.vector.reciprocal(R, Z)
                R64 = work.tile([BH, 1], f32, tag="R64")
                nc.vector.tensor_scalar_mul(R64, R, PISCALE)
                PI = work.tile([BH, S], f32, tag="PI")
                nc.vector.tensor_scalar_mul(PI, E, R64)
                PIs[(h, blk)] = PI

            def make_masks(h, blk):
                PI = PIs[(h, blk)]
                for g in range(2):
                    pa = pst.tile([128, BH], f32, tag="tr")
                    nc.tensor.transpose(
                        pa[0:SG, :], PI[:, g * SG : (g + 1) * SG], identf[0:BH, 0:BH]
                    )
                    nc.vector.tensor_copy(
                        diag_ap(mask8[:, g, :, :], SG, BH), pa[0:SG, :]
                    )

            def vI_u(h, blk):
                """vI psum pair; u = vI + (ques | u1); returns u tile [16, 1024] f32."""
                vp0 = psp.tile([BH, A], f32, tag="pp")
                vp1 = psp.tile([BH, A], f32, tag="pp")
                vps = [vp0, vp1]
                for b in range(BH):
                    for dh in range(2):
                        inb = imgN[(h, blk, b)]
                        nc.tensor.matmul(
                            vps[dh],
                            mask8[:, :, b, :],
                            inb[:, :, dh * A : (dh + 1) * A],
                            start=(b == 0),
                            stop=(b == BH - 1),
                            perf_mode=DR,
                        )
                if blk == 0:
                    u = uh_p.tile([BH, D], f32, tag=f"u1_{h}")
                    other = quesA[h]
                else:
                    u = U1[h]
                    other = u
                for dh in range(2):
                    nc.vector.tensor_add(
                        u[:, dh * A : (dh + 1) * A], vps[dh], other[:, dh * A : (dh + 1) * A]
                    )
                return u

            def u_transpose(u, uT, h, idx):
                for c in range(DC):
                    pt = pst.tile([128, BH], f32, tag="tr")
                    nc.tensor.transpose(pt, u[:, c * 128 : (c + 1) * 128], identf[0:BH, 0:BH])
                    nc.vector.tensor_copy(uT[:, c, h * BH : (h + 1) * BH], pt)

            def qp2(h):
                qp_ps = psp.tile([BH, A], f32, tag="pp")
                for c in range(DC):
                    nc.tensor.matmul(
                        qp_ps,
                        u1T[:, c, h * BH : (h + 1) * BH],
                        wqa2[:, c, :],
                        start=(c == 0),
                        stop=(c == DC - 1),
                    )
                qp = const.tile([BH, A], bf16, tag=f"QP2{h}")
                nc.vector.tensor_add(qp, qp_ps, bqa2b)
                QP2[h] = qp

            def load_imgN(h, blk):
                for b in range(BH):
                    gb = h * BH + b
                    inb = imgn_p.tile([SG, 2, D], f8, tag="imgn")
                    imgN[(h, blk, b)] = inb
                    nc.gpsimd.dma_start(
                        out=inb,
                        in_=img8_h[gb, :, :].rearrange("p (g d) -> p g d", g=2),
                    )

            WF = {}

            def prefetch_fc():
                bfS = const.tile([BH, O], f32)
                nc.gpsimd.dma_start(out=bfS, in_=bcast_ap(bfc_h, BH))
                WF["b"] = bfS
                for n in range(OC):
                    wf = wst.tile([128, DC, ON], bf16, tag=f"wf{n}")
                    nc.sync.dma_start(
                        out=wf,
                        in_=wfc_h[n, :, :].rearrange("p (c x) -> p c x", c=DC),
                    )
                    WF[n] = wf

            def fc(h):
                bfS = WF["b"]
                for n in range(OC):
                    wf = WF[n]
                    fp = psp.tile([BH, ON], f32, tag="pp")
                    for c in range(DC):
                        nc.tensor.matmul(
                            fp,
                            u2T[:, c, h * BH : (h + 1) * BH],
                            wf[:, c, :],
                            start=(c == 0),
                            stop=(c == DC - 1),
                        )
                    sc = work.tile([BH, ON], f32, tag="sc")
                    nc.vector.tensor_add(sc, fp, bfS[:, n * ON : (n + 1) * ON])
                    nc.sync.dma_start(
                        out=score_h[h * BH : (h + 1) * BH, n * ON : (n + 1) * ON],
                        in_=sc,
                    )

            # -------- main schedule: halves interleaved to hide softmax --------
            imgN = {}
            U1 = {}
            QP2 = {}

            load_imgN(0, 0)
            warm(48)
            proj(0, 0)

            load_imgT(1, EARLY_W)
            softmax_pi(0, 0)
            make_masks(0, 0)
            proj(1, 0, 0, 16)  # hides softmax(0,0); needs h1 windows 0-1

            U1[0] = vI_u(0, 0)
            u_transpose(U1[0], u1T, 0, 0)
            qp2(0)
            load_imgN(0, 1)
            proj(0, 1)

            softmax_pi(0, 1)  # issue before wfc hits the sync queue
            load_imgT(1, LATE_W)
            prefetch_fc()
            load_imgN(1, 0)
            proj(1, 0, 16, len(CHUNKS))
            softmax_pi(1, 0, nc.scalar)  # scalar q is idle after proj tanhs
            make_masks(0, 1)

            u2_0 = vI_u(0, 1)
            u_transpose(u2_0, u2T, 0, 2)
            make_masks(1, 0)

            U1[1] = vI_u(1, 0)
            u_transpose(U1[1], u1T, 1, 1)
            qp2(1)
            load_imgN(1, 1)
            proj(1, 1)

            softmax_pi(1, 1, nc.scalar)
            fc(0)  # h0 classifier hides softmax(1,1) round-trip
            make_masks(1, 1)

            u2_1 = vI_u(1, 1)
            u_transpose(u2_1, u2T, 1, 3)
            fc(1)

    nc.compile()
    return nc


def _get_nc():
    global _nc_cache
    if _nc_cache is None:
        _nc_cache = _build_nc()
    return _nc_cache


def _np_cast(x, dt):
    import jax

    x = np.asarray(x)
    if x.nbytes >= 1 << 22:
        # big tensors: multithreaded conversion via jax CPU
        cpu = jax.devices("cpu")[0]
        with jax.default_device(cpu):
            y = jax.jit(lambda v: v.astype(dt), backend="cpu")(x)
            return np.asarray(y)
    return x.astype(dt)


def _to_bf16(x):
    import ml_dtypes

    return _np_cast(x, ml_dtypes.bfloat16)


def _make_in_maps(inputs):
    import ml_dtypes

    bf = ml_dtypes.bfloat16
    f8 = ml_dtypes.float8_e4m3
    ident = np.eye(128)
    selmat = np.zeros((BH, SH), np.float32)
    for b in range(BH):
        selmat[b, b * S : (b + 1) * S] = 1.0
    img_f8 = _np_cast(inputs["img_feat"], f8)
    qp1_full = (
        WSCALE
        * (
            np.asarray(inputs["ques_feat"], np.float32).astype(bf).astype(np.float32)
            @ np.asarray(inputs["W_qa1"], np.float32).astype(bf).astype(np.float32)
            + np.asarray(inputs["b_qa1"], np.float32)
        )
    ).astype(bf)

    def warr(w, dt, scale=1.0):
        # [D, X] -> [128, DC, X] (p, c, x) with d = c*128 + p
        w = np.asarray(w, np.float32) * scale
        x = w.shape[1]
        return np.ascontiguousarray(
            w.reshape(DC, 128, x).transpose(1, 0, 2)
        ).astype(dt)

    shared = {
        "W8_1": warr(inputs["W_ia1"], f8, WSCALE),
        "Wp1": np.asarray(inputs["Wp1"]).astype(bf),
        "W8_2": warr(inputs["W_ia2"], f8, WSCALE),
        "WQA2": warr(inputs["W_qa2"], bf, WSCALE / PISCALE),
        "b_qa2": np.ascontiguousarray(WSCALE * np.asarray(inputs["b_qa2"], np.float32)),
        "Wp2": np.asarray(inputs["Wp2"]).astype(bf),
        "WFC": np.ascontiguousarray(
            warr(inputs["W_fc"], np.float32, 1.0 / PISCALE)
            .reshape(128, DC, OC, ON)
            .transpose(2, 0, 1, 3)
        ).reshape(OC, 128, DC * ON).astype(bf),
        "b_fc": np.ascontiguousarray(inputs["b_fc"], np.float32),
        "SEL": selmat.astype(bf),
        "IDENTF": ident.astype(np.float32),
    }
    in_maps = []
    for c in range(N_CORES):
        sl = slice(c * B, (c + 1) * B)
        m = dict(shared)
        m["img8N"] = np.ascontiguousarray(
            img_f8[sl].reshape(B, 2, SG, D).transpose(0, 2, 1, 3).reshape(B, SG, 2 * D)
        )
        # imgT: (h, p, cc, s) with d = cc*128 + p, split per window contiguous
        imgT = (
            img_f8[sl]
            .reshape(2, SH, D)
            .transpose(0, 2, 1)
            .reshape(2, DC, 128, SH)
            .transpose(0, 2, 1, 3)
        )
        for w, (w0, wl) in enumerate(WINDOWS):
            m[f"imgT8W{w}"] = np.ascontiguousarray(
                imgT[:, :, :, w0 : w0 + wl]
            ).reshape(2, 128, DC * wl)
        m["ques"] = np.ascontiguousarray(
            PISCALE * np.asarray(inputs["ques_feat"], np.float32)[sl]
        )
        m["QP1H"] = qp1_full[sl]
        in_maps.append(m)
    return in_maps


def kernel_run(inputs, trace=False):
    from concourse.bass_utils import run_bass_kernel_spmd

    nc = _get_nc()
    in_maps = _make_in_maps(inputs)
    res = run_bass_kernel_spmd(nc, in_maps, core_ids=list(range(N_CORES)), trace=trace)
    out = np.concatenate([r["score"] for r in res.results], axis=0)
    return out, res


def kernel(**inputs):
    out, _ = kernel_run(inputs)
    return out
